# revision 3
# baseline (speedup 1.0000x reference)
"""Single-head causal attention (B=4, T=4096, C=1024, H=64) on 8 TRN2 cores.

Sharding: 2 cores (folds) per batch element. Slot s of every core owns 512
queries whose causal key range is exactly U_s = 8(s+1) chunks of 128 keys:
fold 0 takes queries [1024s+448, 1024s+960), fold 1 takes the complement
[1024s, 1024s+448) u [1024s+960, 1024s+1024). Both folds' causal boundary
falls in the slot's last 8 key chunks with a slot-independent mask pattern,
so the causal mask is a single per-core constant DMA'd from the host and
the SPMD program is identical on all cores (only input data differs).

Numerics: k/v projection in bf16 with a fused [Wk|Wv] stationary (128 PE
columns); q projection in fp8 DoubleRow with [16Wq|16Wq] (free row-group
duplication; the 16x scale keeps fp8 in its normal range and is undone in
the exp scale). bk cancels in softmax and bv is added on the host, so
neither enters the kernel. Scores are bf16 with the chunk pair split
across disjoint PE row groups; kv storage is parity-swapped (odd key
chunks hold [vT;kT] instead of [kT;vT]) so both stationaries of a pair
come straight out of kv_sb with no duplication DMA. wv uses fp8e4
DoubleRow (256-key contraction) for unmasked pairs and bf16 for the 4
masked pairs per slot (keeping the diagonal band in bf16 is what holds
the overall error at ~1.1e-2). The softmax denominator rides along as a
ones-column of v; outputs leave the chip unnormalized as [65, 512] tiles
and the host transposes, normalizes and adds bv.
"""

import numpy as np
import ml_dtypes

import concourse.bacc as bacc
import concourse.mybir as mybir
from concourse.tile import TileContext
from concourse.masks import make_identity
from concourse.bass_utils import run_bass_kernel_spmd

B, T, C, H = 4, 4096, 1024, 64
P = 128                     # SBUF partitions / key chunk
NB = T // P                 # 32 key chunks
CB = C // P                 # 8 contraction chunks of 128
QS = 512                    # queries per slot
NSLOT = 4
HE = H + 1                  # v extended with a ones column (softmax denom)
HEP = 80                    # DoubleRow stationary width must be 16-aligned
SLOT_U = [8, 16, 24, 32]    # key chunks per slot (uniform across folds)
QSCALE = 16.0               # q projection pre-scale (fp8 range), undone in exp
NCLEAN = 24                 # chunks 0..23 are ever used as clean (fp8) pairs

F32 = mybir.dt.float32
BF16 = mybir.dt.bfloat16
FP8 = mybir.dt.float8e4
BF16NP = ml_dtypes.bfloat16
FP8NP = ml_dtypes.float8_e4m3fn


def _qcols(fold, s):
    if fold == 0:
        return np.arange(1024 * s + 448, 1024 * s + 960)
    return np.concatenate([
        np.arange(1024 * s, 1024 * s + 448),
        np.arange(1024 * s + 960, 1024 * s + 1024),
    ])


def build_bass():
    nc = bacc.Bacc("TRN2", target_bir_lowering=False, debug=False)

    x_kv_d = nc.declare_dram_parameter("x_kv", [T // QS, P, CB, QS], BF16, isOutput=False)
    x_q_d = nc.declare_dram_parameter("x_q", [NSLOT, P, CB // 2, 2, QS], FP8, isOutput=False)
    w2_d = nc.declare_dram_parameter("w2", [P, CB, P], BF16, isOutput=False)
    wq2_d = nc.declare_dram_parameter("wq2", [P, CB // 2, 2, P], FP8, isOutput=False)
    bq2_d = nc.declare_dram_parameter("bq2", [P, 1], F32, isOutput=False)
    mask8_d = nc.declare_dram_parameter("mask8", [P, 8, QS], FP8, isOutput=False)
    out_d = nc.declare_dram_parameter("out", [NSLOT, HE, QS], F32, isOutput=True)

    with TileContext(nc) as tc:
        with (
            tc.tile_pool(name="const", bufs=1) as const,
            tc.tile_pool(name="eb", bufs=4) as eb,
            tc.tile_pool(name="ef", bufs=4) as ef,
            tc.tile_pool(name="wout", bufs=2) as wout,
            tc.tile_pool(name="ps_s", bufs=2, space="PSUM") as ps_s,
            tc.tile_pool(name="ps_o", bufs=1, space="PSUM") as ps_o,
            tc.tile_pool(name="ps_p", bufs=2, space="PSUM") as ps_p,
            tc.tile_pool(name="ps_t", bufs=1, space="PSUM") as ps_t,
        ):
            # ---- persistent SBUF state. All input DMAs are issued up front
            # (each dma_start costs ~650ns of issuing-engine time), spread
            # over four engine queues so issue serialization doesn't gate
            # the first tiles, in first-use order per queue. ----
            xres = const.tile([P, T // QS, CB, QS], BF16, tag="xres")
            xqr = const.tile([P, NSLOT, CB // 2, 2, QS], FP8, tag="xqr")
            mask8_sb = const.tile([P, 8, QS], FP8, tag="mask8")
            w2_sb = const.tile([P, CB, P], BF16, tag="w2")
            wq2_sb = const.tile([P, CB // 2, 2, P], FP8, tag="wq2")
            bq2_sb = const.tile([P, 1], F32, tag="bq2")

            # sync queue: the kv-projection x tiles (tile 0 split in half so
            # the first matmuls can start ~2us earlier)
            nc.sync.dma_start(xres[:, 0, :4, :], x_kv_d[0, :, :4, :])
            nc.sync.dma_start(xres[:, 0, 4:, :], x_kv_d[0, :, 4:, :])
            nc.sync.dma_start(xres[:, 1, :, :], x_kv_d[1])
            for tb in range(4, T // QS):
                nc.sync.dma_start(xres[:, tb, :, :], x_kv_d[tb])
            # scalar queue: weights first (w2 chunk 0-1 gates the very first
            # matmul), then the q-projection constants
            nc.scalar.dma_start(w2_sb[:, :2, :], w2_d[:, :2, :])
            nc.scalar.dma_start(wq2_sb[:], wq2_d[:])
            nc.scalar.dma_start(bq2_sb[:], bq2_d[:])
            nc.scalar.dma_start(w2_sb[:, 2:, :], w2_d[:, 2:, :])
            # gpsimd queue: q-projection x (slot 0 first), two mid-stream kv
            # tiles, mask, rest of x_q
            nc.gpsimd.dma_start(xqr[:, 0], x_q_d[0])
            nc.gpsimd.dma_start(xres[:, 2, :, :], x_kv_d[2])
            nc.gpsimd.dma_start(xres[:, 3, :, :], x_kv_d[3])
            nc.gpsimd.dma_start(mask8_sb[:], mask8_d[:])
            for s in range(1, NSLOT):
                nc.gpsimd.dma_start(xqr[:, s], x_q_d[s])

            id_bf16 = const.tile([P, P], BF16, tag="idb")
            make_identity(nc, id_bf16[:])

            # kv_sb[p, pair, parity, col]: even key chunks store [kT; vT],
            # odd chunks store [vT; kT], so a score pair's two stationaries
            # are kv_sb[:H, i, 0, :] and kv_sb[H:, i, 1, :] — disjoint PE
            # row groups with no duplication DMA. qT rows 64-127 are
            # duplicated for free by the [Wq|Wq] stationary.
            kv_sb = const.tile([P, NB // 2, 2, P], BF16, tag="kv")
            qT_sb = const.tile([P, NSLOT * QS], BF16, tag="qT")
            vext_b = const.tile([P, NB, HE], BF16, tag="vextb")
            nc.vector.memset(vext_b[:, :, H:HE], 1.0)
            vext_f = const.tile([P, NCLEAN // 2, 2, HEP], FP8, tag="vextf")
            nc.vector.memset(vext_f[:, :, :, H:HE], 1.0)
            nc.vector.memset(vext_f[:, :, :, HE:HEP], 0.0)

            def proj_thunks(tb):
                st = {}

                def mk_mm(c):
                    def f():
                        if c == 0:
                            st["pp"] = ps_p.tile([P, 2, 2, P], F32, tag="proj", name="pp")
                        nc.tensor.matmul(
                            st["pp"][:], w2_sb[:, c, :], xres[:, tb, c, :],
                            start=(c == 0), stop=(c == CB - 1),
                        )
                    return f

                def kv_copy():
                    pp = st["pp"]
                    # even chunks (parity 0): straight copy, full 128 rows
                    nc.vector.tensor_copy(
                        kv_sb[:, 2 * tb : 2 * tb + 2, 0, :], pp[:, :, 0, :]
                    )
                    # odd chunks (parity 1): swap halves so kT lands in
                    # rows 64-127 and vT in rows 0-63
                    nc.vector.tensor_copy(
                        kv_sb[H:, 2 * tb : 2 * tb + 2, 1, :], pp[:H, :, 1, :]
                    )
                    nc.vector.tensor_copy(
                        kv_sb[:H, 2 * tb : 2 * tb + 2, 1, :], pp[H:, :, 1, :]
                    )

                return [mk_mm(c) for c in range(CB)] + [kv_copy]

            def vtr_thunks(tb):
                def mk_vtr(sx):
                    def f():
                        tk = tb * (QS // P) + sx
                        par = tk % 2
                        vtp = ps_t.tile([P, H], BF16, tag="tr")
                        if par == 0:
                            vsrc = kv_sb[H:, tk // 2, 0, :]
                            idm = id_bf16[H:, H:]
                        else:
                            vsrc = kv_sb[:H, tk // 2, 1, :]
                            idm = id_bf16[:H, :H]
                        nc.tensor.transpose(vtp[:], vsrc, idm)
                        nc.vector.tensor_copy(vext_b[:, tk, :H], vtp[:])
                        if tk < NCLEAN:
                            nc.vector.tensor_copy(
                                vext_f[:, tk // 2, tk % 2, :H], vtp[:]
                            )
                    return f

                return [mk_vtr(sx) for sx in range(QS // P)]

            def kv_thunks(tb):
                return proj_thunks(tb) + vtr_thunks(tb)

            def q_proj(s):
                qp = ps_p.tile([P, QS], F32, tag="proj")
                for cp in range(CB // 2):
                    nc.tensor.matmul(
                        qp[:], wq2_sb[:, cp, :, :], xqr[:, s, cp, :, :],
                        start=(cp == 0), stop=(cp == CB // 2 - 1),
                        perf_mode=mybir.MatmulPerfMode.DoubleRow,
                    )
                qcols_ = slice(s * QS, (s + 1) * QS)
                nc.vector.tensor_scalar_add(qT_sb[:, qcols_], qp[:], bq2_sb[:])

            # keys 0..1023 (tiles 0,1) must exist before slot 0 attention;
            # q slot 0 right after tile 0 so it overlaps tile 1's DMA
            for th in proj_thunks(0):
                th()
            q_proj(0)
            for th in vtr_thunks(0) + kv_thunks(1):
                th()

            # fill regions (thunks, first pair, deadline pair): each region
            # is dispatched evenly over its global-pair window
            regions = [
                (kv_thunks(2) + kv_thunks(3), 0, 8),
                (kv_thunks(4) + kv_thunks(5), 8, 19),
                (kv_thunks(6) + kv_thunks(7), 19, 31),
            ]
            rfill = [0] * len(regions)

            def run_fill(g):
                for r, (th, g0, g1) in enumerate(regions):
                    if g < g0:
                        continue
                    want = len(th) if g >= g1 else ((g - g0 + 1) * len(th)) // (g1 - g0)
                    while rfill[r] < want:
                        th[rfill[r]]()
                        rfill[r] += 1

            gpair = 0
            for s in range(NSLOT):
                U = SLOT_U[s]
                npairs = U // 2
                qcols = slice(s * QS, (s + 1) * QS)

                oacc = ps_o.tile([HEP, QS], F32, tag="outT")
                pipe = []  # (et, tkp, masked) awaiting their wv matmuls

                def emit_wv(et, tkp, masked, _U=U, _oacc=oacc):
                    first = tkp == 0
                    last = tkp == _U // 2 - 1
                    if masked:
                        for h in range(2):
                            tk = 2 * tkp + h
                            nc.tensor.matmul(
                                _oacc[:HE, :], vext_b[:, tk, :], et[:, h, :],
                                start=(first and h == 0), stop=(last and h == 1),
                            )
                    else:
                        nc.tensor.matmul(
                            _oacc[:], vext_f[:, tkp, :, :], et[:],
                            start=first, stop=last,
                            perf_mode=mybir.MatmulPerfMode.DoubleRow,
                        )

                for tkp in range(npairs):
                    masked = tkp >= npairs - 4
                    sps = ps_s.tile([P, 2, QS], F32, tag="sT")
                    nc.tensor.matmul(
                        sps[:, 0, :], kv_sb[:H, tkp, 0, :], qT_sb[:H, qcols],
                        start=True, stop=True,
                    )
                    nc.tensor.matmul(
                        sps[:, 1, :], kv_sb[H:, tkp, 1, :], qT_sb[H:, qcols],
                        start=True, stop=True,
                    )
                    if masked:
                        et = eb.tile([P, 2, QS], BF16, tag="expb")
                    else:
                        et = ef.tile([P, 2, QS], FP8, tag="expf")
                    nc.scalar.activation(
                        et[:], sps[:], mybir.ActivationFunctionType.Exp,
                        scale=float(H) ** -0.5 / QSCALE,
                    )
                    if masked:
                        mi = 2 * (tkp - (npairs - 4))
                        nc.gpsimd.tensor_tensor(
                            et[:], et[:], mask8_sb[:, mi : mi + 2, :],
                            mybir.AluOpType.mult,
                        )
                    run_fill(gpair)
                    gpair += 1
                    # next slot's q projection early, so the slot boundary
                    # never waits on qT
                    if tkp == 1 and s < NSLOT - 1:
                        q_proj(s + 1)
                    # wv runs one pair behind scores so PE never stalls on ACT
                    pipe.append((et, tkp, masked))
                    if len(pipe) > 1:
                        emit_wv(*pipe.pop(0))
                while pipe:
                    emit_wv(*pipe.pop(0))

                ot = wout.tile([HE, QS], F32, tag="oT")
                nc.vector.tensor_copy(ot[:], oacc[:HE, :])
                nc.gpsimd.dma_start(out_d[s], ot[:])

    nc.compile()
    return nc


_NC_CACHE = None


def _get_nc():
    global _NC_CACHE
    if _NC_CACHE is None:
        _NC_CACHE = build_bass()
    return _NC_CACHE


def _core_inputs(x, Wq, bq, Wk, bk, Wv, bv, b, fold):
    xT = np.asarray(x[b], dtype=np.float32).T          # [C, T]
    x_kv = np.ascontiguousarray(
        xT.reshape(CB, P, T // QS, QS).transpose(2, 1, 0, 3).astype(BF16NP)
    )
    qcols = np.concatenate([_qcols(fold, s) for s in range(NSLOT)])
    xq = xT[:, qcols]                                   # [C, 2048]
    x_q = np.ascontiguousarray(
        xq.reshape(CB // 2, 2, P, NSLOT, QS).transpose(3, 2, 0, 1, 4).astype(FP8NP)
    )
    wk = np.asarray(Wk, np.float32).reshape(CB, P, H)
    wv = np.asarray(Wv, np.float32).reshape(CB, P, H)
    w2 = np.ascontiguousarray(
        np.concatenate([wk, wv], axis=2).transpose(1, 0, 2).astype(BF16NP)
    )
    wqs = (QSCALE * np.asarray(Wq, np.float32)).reshape(CB // 2, 2, P, H)
    wq2 = np.ascontiguousarray(
        np.concatenate([wqs, wqs], axis=3).transpose(2, 0, 1, 3).astype(FP8NP)
    )
    bq2 = np.ascontiguousarray(
        np.tile(QSCALE * np.asarray(bq, np.float32), 2)[:, None]
    )
    p = np.arange(P)[:, None, None]
    i = np.arange(8)[None, :, None]
    j = np.arange(QS)[None, None, :]
    if fold == 0:
        m = (448 + j) >= (128 * i + p)
    else:
        m = np.where(j < 448, j >= (128 * i + p), (512 + j) >= (128 * i + p))
    mask8 = np.ascontiguousarray(m.astype(FP8NP))
    return {
        "x_kv": x_kv,
        "x_q": x_q,
        "w2": w2,
        "wq2": wq2,
        "bq2": bq2,
        "mask8": mask8,
    }


def _assemble(results, bv):
    bvf = np.asarray(bv, np.float32)
    out = np.empty((B, T, H), dtype=np.float32)
    for core in range(8):
        b, fold = core // 2, core % 2
        o = results[core]["out"]                        # [NSLOT, 65, 512]
        for s in range(NSLOT):
            val = (o[s, :H, :] / o[s, H:H + 1, :]).T + bvf
            out[b, _qcols(fold, s), :] = val
    return out


def kernel(x, Wq, bq, Wk, bk, Wv, bv):
    x = np.asarray(x, dtype=np.float32)
    nc = _get_nc()
    core_ids = list(range(8))
    in_maps = [
        _core_inputs(x, Wq, bq, Wk, bk, Wv, bv, core // 2, core % 2)
        for core in core_ids
    ]
    res = run_bass_kernel_spmd(nc, in_maps, core_ids)
    return _assemble(res.results, bv)


# revision 5
# speedup vs baseline: 1.0474x; 1.0474x over previous
"""Single-head causal attention (B=4, T=4096, C=1024, H=64) on 8 TRN2 cores.

Sharding: 2 cores (folds) per batch element. Slot s of every core owns 512
queries whose causal key range is exactly U_s = 8(s+1) chunks of 128 keys:
fold 0 takes queries [1024s+448, 1024s+960), fold 1 takes the complement
[1024s, 1024s+448) u [1024s+960, 1024s+1024). Both folds' causal boundary
falls in the slot's last 8 key chunks with a slot-independent mask pattern,
so the causal mask is a single per-core constant DMA'd from the host and
the SPMD program is identical on all cores (only input data differs).

Numerics: k/v projection in bf16 with a fused [Wk|Wv] stationary (128 PE
columns); q projection in fp8 DoubleRow with [16Wq|16Wq] (free row-group
duplication; the 16x scale keeps fp8 in its normal range and is undone in
the exp scale). bk cancels in softmax and bv is added on the host, so
neither enters the kernel. Scores are bf16 with the chunk pair split
across disjoint PE row groups; kv storage is parity-swapped (odd key
chunks hold [vT;kT] instead of [kT;vT]) so both stationaries of a pair
come straight out of kv_sb with no duplication DMA. wv uses fp8e4
DoubleRow (256-key contraction) for unmasked pairs and bf16 for the 4
masked pairs per slot (keeping the diagonal band in bf16 is what holds
the overall error at ~1.1e-2). The softmax denominator rides along as a
ones-column of v; outputs leave the chip unnormalized as [65, 512] tiles
and the host transposes, normalizes and adds bv.
"""

import numpy as np
import ml_dtypes

import concourse.bacc as bacc
import concourse.mybir as mybir
from concourse.tile import TileContext
from concourse.masks import make_identity
from concourse.bass_utils import run_bass_kernel_spmd

B, T, C, H = 4, 4096, 1024, 64
P = 128                     # SBUF partitions / key chunk
NB = T // P                 # 32 key chunks
CB = C // P                 # 8 contraction chunks of 128
QS = 512                    # queries per slot
NSLOT = 4
HE = H + 1                  # v extended with a ones column (softmax denom)
HEP = 80                    # DoubleRow stationary width must be 16-aligned
SLOT_U = [8, 16, 24, 32]    # key chunks per slot (uniform across folds)
QSCALE = 16.0               # q projection pre-scale (fp8 range), undone in exp
NCLEAN = 24                 # chunks 0..23 are ever used as clean (fp8) pairs

F32 = mybir.dt.float32
BF16 = mybir.dt.bfloat16
FP8 = mybir.dt.float8e4
BF16NP = ml_dtypes.bfloat16
FP8NP = ml_dtypes.float8_e4m3fn


def _qcols(fold, s):
    if fold == 0:
        return np.arange(1024 * s + 448, 1024 * s + 960)
    return np.concatenate([
        np.arange(1024 * s, 1024 * s + 448),
        np.arange(1024 * s + 960, 1024 * s + 1024),
    ])


def build_bass():
    nc = bacc.Bacc("TRN2", target_bir_lowering=False, debug=False)

    x_kv_d = nc.declare_dram_parameter("x_kv", [T // QS, P, CB, QS], BF16, isOutput=False)
    x_q_d = nc.declare_dram_parameter("x_q", [NSLOT, P, CB // 2, 2, QS], FP8, isOutput=False)
    w2_d = nc.declare_dram_parameter("w2", [P, CB, P], BF16, isOutput=False)
    wq2_d = nc.declare_dram_parameter("wq2", [P, CB // 2, 2, P], FP8, isOutput=False)
    bq2_d = nc.declare_dram_parameter("bq2", [P, 1], F32, isOutput=False)
    mask8_d = nc.declare_dram_parameter("mask8", [P, 8, QS], FP8, isOutput=False)
    out_d = nc.declare_dram_parameter("out", [NSLOT, HE, QS], F32, isOutput=True)

    with TileContext(nc) as tc:
        with (
            tc.tile_pool(name="const", bufs=1) as const,
            tc.tile_pool(name="eb", bufs=4) as eb,
            tc.tile_pool(name="ef", bufs=4) as ef,
            tc.tile_pool(name="wout", bufs=2) as wout,
            tc.tile_pool(name="ps_s", bufs=2, space="PSUM") as ps_s,
            tc.tile_pool(name="ps_o", bufs=1, space="PSUM") as ps_o,
            tc.tile_pool(name="ps_p", bufs=2, space="PSUM") as ps_p,
            tc.tile_pool(name="ps_t", bufs=1, space="PSUM") as ps_t,
        ):
            # ---- persistent SBUF state. All input DMAs are issued up front
            # (each dma_start costs ~650ns of issuing-engine time), spread
            # over four engine queues so issue serialization doesn't gate
            # the first tiles, in first-use order per queue. ----
            xres = const.tile([P, T // QS, CB, QS], BF16, tag="xres")
            xqr = const.tile([P, NSLOT, CB // 2, 2, QS], FP8, tag="xqr")
            mask8_sb = const.tile([P, 8, QS], FP8, tag="mask8")
            w2_sb = const.tile([P, CB, P], BF16, tag="w2")
            wq2_sb = const.tile([P, CB // 2, 2, P], FP8, tag="wq2")
            bq2_sb = const.tile([P, 1], F32, tag="bq2")

            # sync queue: the kv-projection x tiles (tile 0 split in half so
            # the first matmuls can start ~2us earlier)
            nc.sync.dma_start(xres[:, 0, :4, :], x_kv_d[0, :, :4, :])
            nc.sync.dma_start(xres[:, 0, 4:, :], x_kv_d[0, :, 4:, :])
            nc.sync.dma_start(xres[:, 1, :, :], x_kv_d[1])
            for tb in range(4, T // QS):
                nc.sync.dma_start(xres[:, tb, :, :], x_kv_d[tb])
            # scalar queue: weights first (w2 chunk 0-1 gates the very first
            # matmul), then the q-projection constants
            nc.scalar.dma_start(w2_sb[:, :2, :], w2_d[:, :2, :])
            nc.scalar.dma_start(wq2_sb[:], wq2_d[:])
            nc.scalar.dma_start(bq2_sb[:], bq2_d[:])
            nc.scalar.dma_start(w2_sb[:, 2:, :], w2_d[:, 2:, :])
            # gpsimd queue: mask (small, needed by the first masked pair),
            # q-projection x (slot 0), two mid-stream kv tiles, rest of x_q
            nc.gpsimd.dma_start(mask8_sb[:], mask8_d[:])
            nc.gpsimd.dma_start(xqr[:, 0], x_q_d[0])
            nc.gpsimd.dma_start(xres[:, 2, :, :], x_kv_d[2])
            nc.gpsimd.dma_start(xres[:, 3, :, :], x_kv_d[3])
            for s in range(1, NSLOT):
                nc.gpsimd.dma_start(xqr[:, s], x_q_d[s])

            id_bf16 = const.tile([P, P], BF16, tag="idb")
            make_identity(nc, id_bf16[:])

            # kv_sb[p, pair, parity, col]: even key chunks store [kT; vT],
            # odd chunks store [vT; kT], so a score pair's two stationaries
            # are kv_sb[:H, i, 0, :] and kv_sb[H:, i, 1, :] — disjoint PE
            # row groups with no duplication DMA. qT rows 64-127 are
            # duplicated for free by the [Wq|Wq] stationary.
            kv_sb = const.tile([P, NB // 2, 2, P], BF16, tag="kv")
            qT_sb = const.tile([P, NSLOT * QS], BF16, tag="qT")
            vext_b = const.tile([P, NB, HE], BF16, tag="vextb")
            nc.vector.memset(vext_b[:, :, H:HE], 1.0)
            vext_f = const.tile([P, NCLEAN // 2, 2, HEP], FP8, tag="vextf")
            nc.vector.memset(vext_f[:, :, :, H:HE], 1.0)
            nc.vector.memset(vext_f[:, :, :, HE:HEP], 0.0)

            def proj_thunks(tb):
                st = {}

                def mk_mm(c):
                    def f():
                        if c == 0:
                            st["pp"] = ps_p.tile([P, 2, 2, P], F32, tag="proj", name="pp")
                        nc.tensor.matmul(
                            st["pp"][:], w2_sb[:, c, :], xres[:, tb, c, :],
                            start=(c == 0), stop=(c == CB - 1),
                        )
                    return f

                def kv_copy():
                    pp = st["pp"]
                    # even chunks (parity 0): straight copy, full 128 rows
                    nc.vector.tensor_copy(
                        kv_sb[:, 2 * tb : 2 * tb + 2, 0, :], pp[:, :, 0, :]
                    )
                    # odd chunks (parity 1): swap halves so kT lands in
                    # rows 64-127 and vT in rows 0-63
                    nc.vector.tensor_copy(
                        kv_sb[H:, 2 * tb : 2 * tb + 2, 1, :], pp[:H, :, 1, :]
                    )
                    nc.vector.tensor_copy(
                        kv_sb[:H, 2 * tb : 2 * tb + 2, 1, :], pp[H:, :, 1, :]
                    )

                return [mk_mm(c) for c in range(CB)] + [kv_copy]

            def vtr_thunks(tb):
                def mk_vtr(sx):
                    def f():
                        tk = tb * (QS // P) + sx
                        par = tk % 2
                        vtp = ps_t.tile([P, H], BF16, tag="tr")
                        if par == 0:
                            vsrc = kv_sb[H:, tk // 2, 0, :]
                            idm = id_bf16[H:, H:]
                        else:
                            vsrc = kv_sb[:H, tk // 2, 1, :]
                            idm = id_bf16[:H, :H]
                        nc.tensor.transpose(vtp[:], vsrc, idm)
                        nc.vector.tensor_copy(vext_b[:, tk, :H], vtp[:])
                        if tk < NCLEAN:
                            nc.vector.tensor_copy(
                                vext_f[:, tk // 2, tk % 2, :H], vtp[:]
                            )
                    return f

                return [mk_vtr(sx) for sx in range(QS // P)]

            def kv_thunks(tb):
                return proj_thunks(tb) + vtr_thunks(tb)

            def q_proj(s):
                qp = ps_p.tile([P, QS], F32, tag="proj")
                for cp in range(CB // 2):
                    nc.tensor.matmul(
                        qp[:], wq2_sb[:, cp, :, :], xqr[:, s, cp, :, :],
                        start=(cp == 0), stop=(cp == CB // 2 - 1),
                        perf_mode=mybir.MatmulPerfMode.DoubleRow,
                    )
                qcols_ = slice(s * QS, (s + 1) * QS)
                nc.vector.tensor_scalar_add(qT_sb[:, qcols_], qp[:], bq2_sb[:])

            # keys 0..1023 (tiles 0,1) must exist before slot 0 attention;
            # q slot 0 right after tile 0 so it overlaps tile 1's DMA
            for th in proj_thunks(0):
                th()
            q_proj(0)
            for th in vtr_thunks(0) + kv_thunks(1):
                th()

            # fill regions (thunks, first pair, deadline pair): each region
            # is dispatched evenly over its global-pair window
            regions = [
                (kv_thunks(2) + kv_thunks(3), 0, 8),
                (kv_thunks(4) + kv_thunks(5), 8, 19),
                (kv_thunks(6) + kv_thunks(7), 19, 31),
            ]
            rfill = [0] * len(regions)

            def run_fill(g):
                for r, (th, g0, g1) in enumerate(regions):
                    if g < g0:
                        continue
                    want = len(th) if g >= g1 else ((g - g0 + 1) * len(th)) // (g1 - g0)
                    while rfill[r] < want:
                        th[rfill[r]]()
                        rfill[r] += 1

            gpair = 0
            for s in range(NSLOT):
                U = SLOT_U[s]
                npairs = U // 2
                qcols = slice(s * QS, (s + 1) * QS)

                oacc = ps_o.tile([HEP, QS], F32, tag="outT")
                pipe = []  # (et, tkp, masked) awaiting their wv matmuls

                def emit_wv(et, tkp, masked, _U=U, _oacc=oacc):
                    first = tkp == 0
                    last = tkp == _U // 2 - 1
                    if masked:
                        for h in range(2):
                            tk = 2 * tkp + h
                            nc.tensor.matmul(
                                _oacc[:HE, :], vext_b[:, tk, :], et[:, h, :],
                                start=(first and h == 0), stop=(last and h == 1),
                            )
                    else:
                        nc.tensor.matmul(
                            _oacc[:], vext_f[:, tkp, :, :], et[:],
                            start=first, stop=last,
                            perf_mode=mybir.MatmulPerfMode.DoubleRow,
                        )

                for tkp in range(npairs):
                    masked = tkp >= npairs - 4
                    sps = ps_s.tile([P, 2, QS], F32, tag="sT")
                    nc.tensor.matmul(
                        sps[:, 0, :], kv_sb[:H, tkp, 0, :], qT_sb[:H, qcols],
                        start=True, stop=True,
                    )
                    nc.tensor.matmul(
                        sps[:, 1, :], kv_sb[H:, tkp, 1, :], qT_sb[H:, qcols],
                        start=True, stop=True,
                    )
                    if masked:
                        et = eb.tile([P, 2, QS], BF16, tag="expb")
                    else:
                        et = ef.tile([P, 2, QS], FP8, tag="expf")
                    nc.scalar.activation(
                        et[:], sps[:], mybir.ActivationFunctionType.Exp,
                        scale=float(H) ** -0.5 / QSCALE,
                    )
                    if masked:
                        mi = 2 * (tkp - (npairs - 4))
                        nc.vector.tensor_tensor(
                            et[:], et[:], mask8_sb[:, mi : mi + 2, :],
                            mybir.AluOpType.mult,
                        )
                    run_fill(gpair)
                    gpair += 1
                    # next slot's q projection early, so the slot boundary
                    # never waits on qT
                    if tkp == 1 and s < NSLOT - 1:
                        q_proj(s + 1)
                    # wv runs one pair behind scores so PE never stalls on ACT
                    pipe.append((et, tkp, masked))
                    if len(pipe) > 1:
                        emit_wv(*pipe.pop(0))
                while pipe:
                    emit_wv(*pipe.pop(0))

                ot = wout.tile([HE, QS], F32, tag="oT")
                nc.vector.tensor_copy(ot[:], oacc[:HE, :])
                nc.gpsimd.dma_start(out_d[s], ot[:])

    nc.compile()
    return nc


_NC_CACHE = None


def _get_nc():
    global _NC_CACHE
    if _NC_CACHE is None:
        _NC_CACHE = build_bass()
    return _NC_CACHE


def _core_inputs(x, Wq, bq, Wk, bk, Wv, bv, b, fold):
    xT = np.asarray(x[b], dtype=np.float32).T          # [C, T]
    x_kv = np.ascontiguousarray(
        xT.reshape(CB, P, T // QS, QS).transpose(2, 1, 0, 3).astype(BF16NP)
    )
    qcols = np.concatenate([_qcols(fold, s) for s in range(NSLOT)])
    xq = xT[:, qcols]                                   # [C, 2048]
    x_q = np.ascontiguousarray(
        xq.reshape(CB // 2, 2, P, NSLOT, QS).transpose(3, 2, 0, 1, 4).astype(FP8NP)
    )
    wk = np.asarray(Wk, np.float32).reshape(CB, P, H)
    wv = np.asarray(Wv, np.float32).reshape(CB, P, H)
    w2 = np.ascontiguousarray(
        np.concatenate([wk, wv], axis=2).transpose(1, 0, 2).astype(BF16NP)
    )
    wqs = (QSCALE * np.asarray(Wq, np.float32)).reshape(CB // 2, 2, P, H)
    wq2 = np.ascontiguousarray(
        np.concatenate([wqs, wqs], axis=3).transpose(2, 0, 1, 3).astype(FP8NP)
    )
    bq2 = np.ascontiguousarray(
        np.tile(QSCALE * np.asarray(bq, np.float32), 2)[:, None]
    )
    p = np.arange(P)[:, None, None]
    i = np.arange(8)[None, :, None]
    j = np.arange(QS)[None, None, :]
    if fold == 0:
        m = (448 + j) >= (128 * i + p)
    else:
        m = np.where(j < 448, j >= (128 * i + p), (512 + j) >= (128 * i + p))
    mask8 = np.ascontiguousarray(m.astype(FP8NP))
    return {
        "x_kv": x_kv,
        "x_q": x_q,
        "w2": w2,
        "wq2": wq2,
        "bq2": bq2,
        "mask8": mask8,
    }


def _assemble(results, bv):
    bvf = np.asarray(bv, np.float32)
    out = np.empty((B, T, H), dtype=np.float32)
    for core in range(8):
        b, fold = core // 2, core % 2
        o = results[core]["out"]                        # [NSLOT, 65, 512]
        for s in range(NSLOT):
            val = (o[s, :H, :] / o[s, H:H + 1, :]).T + bvf
            out[b, _qcols(fold, s), :] = val
    return out


def kernel(x, Wq, bq, Wk, bk, Wv, bv):
    x = np.asarray(x, dtype=np.float32)
    nc = _get_nc()
    core_ids = list(range(8))
    in_maps = [
        _core_inputs(x, Wq, bq, Wk, bk, Wv, bv, core // 2, core % 2)
        for core in core_ids
    ]
    res = run_bass_kernel_spmd(nc, in_maps, core_ids)
    return _assemble(res.results, bv)


# revision 9
# speedup vs baseline: 1.1041x; 1.0541x over previous
"""Single-head causal attention (B=4, T=4096, C=1024, H=64) on 8 TRN2 cores.

Sharding: 2 cores (folds) per batch element. Slot s of every core owns 512
queries whose causal key range is exactly U_s = 8(s+1) chunks of 128 keys:
fold 0 takes queries [1024s+448, 1024s+960), fold 1 takes the complement
[1024s, 1024s+448) u [1024s+960, 1024s+1024). Both folds' causal boundary
falls in the slot's last 8 key chunks with a slot-independent mask pattern,
so the causal mask is a single per-core constant DMA'd from the host and
the SPMD program is identical on all cores (only input data differs).

Numerics: k/v projection in bf16 with a fused [Wk|Wv] stationary (128 PE
columns); q projection in fp8 DoubleRow with [16Wq|16Wq] (free row-group
duplication; the 16x scale keeps fp8 in its normal range and is undone in
the exp scale). bk cancels in softmax and bv is added on the host, so
neither enters the kernel. Scores are bf16 with the chunk pair split
across disjoint PE row groups; kv storage is parity-swapped (odd key
chunks hold [vT;kT] instead of [kT;vT]) so both stationaries of a pair
come straight out of kv_sb with no duplication DMA. wv uses fp8e4
DoubleRow (256-key contraction) for unmasked pairs and bf16 for the 4
masked pairs per slot (keeping the diagonal band in bf16 is what holds
the overall error at ~1.1e-2). The softmax denominator rides along as a
ones-column of v; outputs leave the chip unnormalized as [65, 512] tiles
and the host transposes, normalizes and adds bv.
"""

import numpy as np
import ml_dtypes

import concourse.bacc as bacc
import concourse.mybir as mybir
from concourse.tile import TileContext
from concourse.masks import make_identity
from concourse.bass_utils import run_bass_kernel_spmd

B, T, C, H = 4, 4096, 1024, 64
P = 128                     # SBUF partitions / key chunk
NB = T // P                 # 32 key chunks
CB = C // P                 # 8 contraction chunks of 128
QS = 512                    # queries per slot
NSLOT = 4
HE = H + 1                  # v extended with a ones column (softmax denom)
HEP = 80                    # DoubleRow stationary width must be 16-aligned
SLOT_U = [8, 16, 24, 32]    # key chunks per slot (uniform across folds)
QSCALE = 16.0               # q projection pre-scale (fp8 range), undone in exp
NCLEAN = 24                 # chunks 0..23 are ever used as clean (fp8) pairs

F32 = mybir.dt.float32
BF16 = mybir.dt.bfloat16
FP8 = mybir.dt.float8e4
BF16NP = ml_dtypes.bfloat16
FP8NP = ml_dtypes.float8_e4m3fn


def _qcols(fold, s):
    if fold == 0:
        return np.arange(1024 * s + 448, 1024 * s + 960)
    return np.concatenate([
        np.arange(1024 * s, 1024 * s + 448),
        np.arange(1024 * s + 960, 1024 * s + 1024),
    ])


def build_bass():
    nc = bacc.Bacc("TRN2", target_bir_lowering=False, debug=False)

    x_kv_d = nc.declare_dram_parameter("x_kv", [T // QS, P, CB, QS], BF16, isOutput=False)
    x_q_d = nc.declare_dram_parameter("x_q", [NSLOT, P, CB // 2, 2, QS], FP8, isOutput=False)
    w2_d = nc.declare_dram_parameter("w2", [P, CB, P], BF16, isOutput=False)
    wq2_d = nc.declare_dram_parameter("wq2", [P, CB // 2, 2, P], FP8, isOutput=False)
    bq2_d = nc.declare_dram_parameter("bq2", [P, 1], F32, isOutput=False)
    mask8_d = nc.declare_dram_parameter("mask8", [P, 8, QS], BF16, isOutput=False)
    out_d = nc.declare_dram_parameter("out", [NSLOT, HE, QS], F32, isOutput=True)

    with TileContext(nc) as tc:
        with (
            tc.tile_pool(name="const", bufs=1) as const,
            tc.tile_pool(name="eb", bufs=4) as eb,
            tc.tile_pool(name="ef", bufs=4) as ef,
            tc.tile_pool(name="wout", bufs=2) as wout,
            tc.tile_pool(name="ps_s", bufs=2, space="PSUM") as ps_s,
            tc.tile_pool(name="ps_o", bufs=1, space="PSUM") as ps_o,
            tc.tile_pool(name="ps_p", bufs=2, space="PSUM") as ps_p,
            tc.tile_pool(name="ps_t", bufs=1, space="PSUM") as ps_t,
        ):
            # ---- persistent SBUF state. All input DMAs are issued up front
            # (each dma_start costs ~650ns of issuing-engine time), spread
            # over four engine queues so issue serialization doesn't gate
            # the first tiles, in first-use order per queue. ----
            xres = const.tile([P, T // QS, CB, QS], BF16, tag="xres")
            xqr = const.tile([P, NSLOT, CB // 2, 2, QS], FP8, tag="xqr")
            mask8_sb = const.tile([P, 8, QS], BF16, tag="mask8")
            w2_sb = const.tile([P, CB, P], BF16, tag="w2")
            wq2_sb = const.tile([P, CB // 2, 2, P], FP8, tag="wq2")
            bq2_sb = const.tile([P, 1], F32, tag="bq2")

            # sync queue: the kv-projection x tiles (tile 0 split in half so
            # the first matmuls can start ~2us earlier)
            nc.sync.dma_start(xres[:, 0, :4, :], x_kv_d[0, :, :4, :])
            nc.sync.dma_start(xres[:, 0, 4:, :], x_kv_d[0, :, 4:, :])
            nc.sync.dma_start(xres[:, 1, :, :], x_kv_d[1])
            for tb in range(4, T // QS):
                nc.sync.dma_start(xres[:, tb, :, :], x_kv_d[tb])
            # scalar queue: all the small weights, ahead of everything (ring
            # FIFO order follows issue order — small critical transfers must
            # precede the big x streams)
            nc.scalar.dma_start(w2_sb[:], w2_d[:])
            nc.scalar.dma_start(wq2_sb[:], wq2_d[:])
            nc.scalar.dma_start(bq2_sb[:], bq2_d[:])
            # gpsimd queue: q-projection x (slot 0), mask, two mid-stream kv
            # tiles, rest of x_q
            nc.gpsimd.dma_start(xqr[:, 0], x_q_d[0])
            nc.gpsimd.dma_start(mask8_sb[:], mask8_d[:])
            nc.gpsimd.dma_start(xres[:, 2, :, :], x_kv_d[2])
            nc.gpsimd.dma_start(xres[:, 3, :, :], x_kv_d[3])
            for s in range(1, NSLOT):
                nc.gpsimd.dma_start(xqr[:, s], x_q_d[s])

            id_bf16 = const.tile([P, P], BF16, tag="idb")
            make_identity(nc, id_bf16[:])

            # kv_sb[p, pair, parity, col]: even key chunks store [kT; vT],
            # odd chunks store [vT; kT], so a score pair's two stationaries
            # are kv_sb[:H, i, 0, :] and kv_sb[H:, i, 1, :] — disjoint PE
            # row groups with no duplication DMA. qT rows 64-127 are
            # duplicated for free by the [Wq|Wq] stationary.
            kv_sb = const.tile([P, NB // 2, 2, P], BF16, tag="kv")
            qT_sb = const.tile([P, NSLOT * QS], BF16, tag="qT")
            vext_b = const.tile([P, NB, HE], BF16, tag="vextb")
            nc.vector.memset(vext_b[:, :, H:HE], 1.0)
            vext_f = const.tile([P, NCLEAN // 2, 2, HEP], FP8, tag="vextf")
            nc.vector.memset(vext_f[:, :, :, H:HE], 1.0)
            nc.vector.memset(vext_f[:, :, :, HE:HEP], 0.0)

            def proj_thunks(tb):
                st = {}

                def mk_mm(c):
                    def f():
                        if c == 0:
                            st["pp"] = ps_p.tile([P, 2, 2, P], F32, tag="proj", name="pp")
                        nc.tensor.matmul(
                            st["pp"][:], w2_sb[:, c, :], xres[:, tb, c, :],
                            start=(c == 0), stop=(c == CB - 1),
                        )
                    return f

                def kv_copy():
                    pp = st["pp"]
                    # even chunks (parity 0): straight copy, full 128 rows
                    nc.vector.tensor_copy(
                        kv_sb[:, 2 * tb : 2 * tb + 2, 0, :], pp[:, :, 0, :]
                    )
                    # odd chunks (parity 1): swap halves so kT lands in
                    # rows 64-127 and vT in rows 0-63
                    nc.vector.tensor_copy(
                        kv_sb[H:, 2 * tb : 2 * tb + 2, 1, :], pp[:H, :, 1, :]
                    )
                    nc.vector.tensor_copy(
                        kv_sb[:H, 2 * tb : 2 * tb + 2, 1, :], pp[H:, :, 1, :]
                    )

                return [mk_mm(c) for c in range(CB)] + [kv_copy]

            def vtr_thunks(tb):
                def mk_vtr(sx):
                    def f():
                        tk = tb * (QS // P) + sx
                        par = tk % 2
                        vtp = ps_t.tile([P, H], BF16, tag="tr")
                        if par == 0:
                            vsrc = kv_sb[H:, tk // 2, 0, :]
                            idm = id_bf16[H:, H:]
                        else:
                            vsrc = kv_sb[:H, tk // 2, 1, :]
                            idm = id_bf16[:H, :H]
                        nc.tensor.transpose(vtp[:], vsrc, idm)
                        nc.vector.tensor_copy(vext_b[:, tk, :H], vtp[:])
                        if tk < NCLEAN:
                            nc.vector.tensor_copy(
                                vext_f[:, tk // 2, tk % 2, :H], vtp[:]
                            )
                    return f

                return [mk_vtr(sx) for sx in range(QS // P)]

            def kv_thunks(tb):
                return proj_thunks(tb) + vtr_thunks(tb)

            def q_proj(s):
                qp = ps_p.tile([P, QS], F32, tag="proj")
                for cp in range(CB // 2):
                    nc.tensor.matmul(
                        qp[:], wq2_sb[:, cp, :, :], xqr[:, s, cp, :, :],
                        start=(cp == 0), stop=(cp == CB // 2 - 1),
                        perf_mode=mybir.MatmulPerfMode.DoubleRow,
                    )
                qcols_ = slice(s * QS, (s + 1) * QS)
                nc.vector.tensor_scalar_add(qT_sb[:, qcols_], qp[:], bq2_sb[:])

            # keys 0..1023 (tiles 0,1) must exist before slot 0 attention;
            # q slot 0 right after tile 0 so it overlaps tile 1's DMA
            for th in proj_thunks(0):
                th()
            q_proj(0)
            for th in vtr_thunks(0) + kv_thunks(1):
                th()

            # fill regions (thunks, first pair, deadline pair): each region
            # is dispatched evenly over its global-pair window
            regions = [
                (kv_thunks(2) + kv_thunks(3), 0, 8),
                (kv_thunks(4) + kv_thunks(5), 8, 19),
                (kv_thunks(6) + kv_thunks(7), 19, 31),
            ]
            rfill = [0] * len(regions)

            def run_fill(g):
                for r, (th, g0, g1) in enumerate(regions):
                    if g < g0:
                        continue
                    want = len(th) if g >= g1 else ((g - g0 + 1) * len(th)) // (g1 - g0)
                    while rfill[r] < want:
                        th[rfill[r]]()
                        rfill[r] += 1

            gpair = 0
            for s in range(NSLOT):
                U = SLOT_U[s]
                npairs = U // 2
                qcols = slice(s * QS, (s + 1) * QS)

                oacc = ps_o.tile([HEP, QS], F32, tag="outT")
                pipe = []  # (et, tkp, masked) awaiting their wv matmuls

                def emit_wv(et, tkp, masked, _U=U, _oacc=oacc):
                    first = tkp == 0
                    last = tkp == _U // 2 - 1
                    if masked:
                        for h in range(2):
                            tk = 2 * tkp + h
                            nc.tensor.matmul(
                                _oacc[:HE, :], vext_b[:, tk, :], et[:, h, :],
                                start=(first and h == 0), stop=(last and h == 1),
                            )
                    else:
                        nc.tensor.matmul(
                            _oacc[:], vext_f[:, tkp, :, :], et[:],
                            start=first, stop=last,
                            perf_mode=mybir.MatmulPerfMode.DoubleRow,
                        )

                for tkp in range(npairs):
                    masked = tkp >= npairs - 4
                    sps = ps_s.tile([P, 2, QS], F32, tag="sT")
                    nc.tensor.matmul(
                        sps[:, 0, :], kv_sb[:H, tkp, 0, :], qT_sb[:H, qcols],
                        start=True, stop=True,
                    )
                    nc.tensor.matmul(
                        sps[:, 1, :], kv_sb[H:, tkp, 1, :], qT_sb[H:, qcols],
                        start=True, stop=True,
                    )
                    if masked:
                        et = eb.tile([P, 2, QS], BF16, tag="expb")
                    else:
                        et = ef.tile([P, 2, QS], FP8, tag="expf")
                    nc.scalar.activation(
                        et[:], sps[:], mybir.ActivationFunctionType.Exp,
                        scale=float(H) ** -0.5 / QSCALE,
                    )
                    if masked:
                        mi = 2 * (tkp - (npairs - 4))
                        nc.vector.tensor_tensor(
                            et[:], et[:], mask8_sb[:, mi : mi + 2, :],
                            mybir.AluOpType.mult,
                        )
                    run_fill(gpair)
                    gpair += 1
                    # next slot's q projection early, so the slot boundary
                    # never waits on qT
                    if tkp == 1 and s < NSLOT - 1:
                        q_proj(s + 1)
                    # wv runs one pair behind scores so PE never stalls on ACT
                    pipe.append((et, tkp, masked))
                    if len(pipe) > 1:
                        emit_wv(*pipe.pop(0))
                while pipe:
                    emit_wv(*pipe.pop(0))

                ot = wout.tile([HE, QS], F32, tag="oT")
                nc.vector.tensor_copy(ot[:], oacc[:HE, :])
                nc.gpsimd.dma_start(out_d[s], ot[:])

    nc.compile()
    return nc


_NC_CACHE = None


def _get_nc():
    global _NC_CACHE
    if _NC_CACHE is None:
        _NC_CACHE = build_bass()
    return _NC_CACHE


def _core_inputs(x, Wq, bq, Wk, bk, Wv, bv, b, fold):
    xT = np.asarray(x[b], dtype=np.float32).T          # [C, T]
    x_kv = np.ascontiguousarray(
        xT.reshape(CB, P, T // QS, QS).transpose(2, 1, 0, 3).astype(BF16NP)
    )
    qcols = np.concatenate([_qcols(fold, s) for s in range(NSLOT)])
    xq = xT[:, qcols]                                   # [C, 2048]
    x_q = np.ascontiguousarray(
        xq.reshape(CB // 2, 2, P, NSLOT, QS).transpose(3, 2, 0, 1, 4).astype(FP8NP)
    )
    wk = np.asarray(Wk, np.float32).reshape(CB, P, H)
    wv = np.asarray(Wv, np.float32).reshape(CB, P, H)
    w2 = np.ascontiguousarray(
        np.concatenate([wk, wv], axis=2).transpose(1, 0, 2).astype(BF16NP)
    )
    wqs = (QSCALE * np.asarray(Wq, np.float32)).reshape(CB // 2, 2, P, H)
    wq2 = np.ascontiguousarray(
        np.concatenate([wqs, wqs], axis=3).transpose(2, 0, 1, 3).astype(FP8NP)
    )
    bq2 = np.ascontiguousarray(
        np.tile(QSCALE * np.asarray(bq, np.float32), 2)[:, None]
    )
    p = np.arange(P)[:, None, None]
    i = np.arange(8)[None, :, None]
    j = np.arange(QS)[None, None, :]
    if fold == 0:
        m = (448 + j) >= (128 * i + p)
    else:
        m = np.where(j < 448, j >= (128 * i + p), (512 + j) >= (128 * i + p))
    mask8 = np.ascontiguousarray(m.astype(BF16NP))
    return {
        "x_kv": x_kv,
        "x_q": x_q,
        "w2": w2,
        "wq2": wq2,
        "bq2": bq2,
        "mask8": mask8,
    }


def _assemble(results, bv):
    bvf = np.asarray(bv, np.float32)
    out = np.empty((B, T, H), dtype=np.float32)
    for core in range(8):
        b, fold = core // 2, core % 2
        o = results[core]["out"]                        # [NSLOT, 65, 512]
        for s in range(NSLOT):
            val = (o[s, :H, :] / o[s, H:H + 1, :]).T + bvf
            out[b, _qcols(fold, s), :] = val
    return out


def kernel(x, Wq, bq, Wk, bk, Wv, bv):
    x = np.asarray(x, dtype=np.float32)
    nc = _get_nc()
    core_ids = list(range(8))
    in_maps = [
        _core_inputs(x, Wq, bq, Wk, bk, Wv, bv, core // 2, core % 2)
        for core in core_ids
    ]
    res = run_bass_kernel_spmd(nc, in_maps, core_ids)
    return _assemble(res.results, bv)


# revision 11
# speedup vs baseline: 1.2143x; 1.0998x over previous
"""Single-head causal attention (B=4, T=4096, C=1024, H=64) on 8 TRN2 cores.

Sharding: 2 cores (folds) per batch element. Slot s of every core owns 512
queries whose causal key range is exactly U_s = 8(s+1) chunks of 128 keys:
fold 0 takes queries [1024s+448, 1024s+960), fold 1 takes the complement
[1024s, 1024s+448) u [1024s+960, 1024s+1024). Both folds' causal boundary
falls in the slot's last 8 key chunks with a slot-independent mask pattern,
so the causal mask is a single per-core constant DMA'd from the host and
the SPMD program is identical on all cores (only input data differs).

Numerics: k/v projection in bf16 with a fused [Wk|Wv] stationary (128 PE
columns); q projection in fp8 DoubleRow with [16Wq|16Wq] (free row-group
duplication; the 16x scale keeps fp8 in its normal range and is undone in
the exp scale). bk cancels in softmax and bv is added on the host, so
neither enters the kernel. Scores are bf16 with the chunk pair split
across disjoint PE row groups; kv storage is parity-swapped (odd key
chunks hold [vT;kT] instead of [kT;vT]) so both stationaries of a pair
come straight out of kv_sb with no duplication DMA. wv uses fp8e4
DoubleRow (256-key contraction) for unmasked pairs and bf16 for the 4
masked pairs per slot (keeping the diagonal band in bf16 is what holds
the overall error at ~1.1e-2). The softmax denominator rides along as a
ones-column of v; outputs leave the chip unnormalized as [65, 512] tiles
and the host transposes, normalizes and adds bv.
"""

import numpy as np
import ml_dtypes

import concourse.bacc as bacc
import concourse.mybir as mybir
from concourse.tile import TileContext
from concourse.masks import make_identity
from concourse.bass_utils import run_bass_kernel_spmd

B, T, C, H = 4, 4096, 1024, 64
P = 128                     # SBUF partitions / key chunk
NB = T // P                 # 32 key chunks
CB = C // P                 # 8 contraction chunks of 128
QS = 512                    # queries per slot
NSLOT = 4
HE = H + 1                  # v extended with a ones column (softmax denom)
HEP = 80                    # DoubleRow stationary width must be 16-aligned
SLOT_U = [8, 16, 24, 32]    # key chunks per slot (uniform across folds)
QSCALE = 16.0               # q projection pre-scale (fp8 range), undone in exp
NCLEAN = 24                 # chunks 0..23 are ever used as clean (fp8) pairs

F32 = mybir.dt.float32
BF16 = mybir.dt.bfloat16
FP8 = mybir.dt.float8e4
BF16NP = ml_dtypes.bfloat16
FP8NP = ml_dtypes.float8_e4m3fn


def _qcols(fold, s):
    if fold == 0:
        return np.arange(1024 * s + 448, 1024 * s + 960)
    return np.concatenate([
        np.arange(1024 * s, 1024 * s + 448),
        np.arange(1024 * s + 960, 1024 * s + 1024),
    ])


def build_bass():
    nc = bacc.Bacc("TRN2", target_bir_lowering=False, debug=False)

    x_kv_d = nc.declare_dram_parameter("x_kv", [T // QS, P, CB, QS], BF16, isOutput=False)
    x_q_d = nc.declare_dram_parameter("x_q", [NSLOT, P, CB // 2, 2, QS], FP8, isOutput=False)
    w2_d = nc.declare_dram_parameter("w2", [P, CB, P], BF16, isOutput=False)
    wq2_d = nc.declare_dram_parameter("wq2", [P, CB // 2, 2, P], FP8, isOutput=False)
    bq2_d = nc.declare_dram_parameter("bq2", [P, 1], F32, isOutput=False)
    mask8_d = nc.declare_dram_parameter("mask8", [P, 8, QS], BF16, isOutput=False)
    out_d = nc.declare_dram_parameter("out", [NSLOT, HE, QS], F32, isOutput=True)

    with TileContext(nc) as tc:
        with (
            tc.tile_pool(name="const", bufs=1) as const,
            tc.tile_pool(name="eb", bufs=4) as eb,
            tc.tile_pool(name="ef", bufs=4) as ef,
            tc.tile_pool(name="wout", bufs=2) as wout,
            tc.tile_pool(name="ps_s", bufs=2, space="PSUM") as ps_s,
            tc.tile_pool(name="ps_o", bufs=1, space="PSUM") as ps_o,
            tc.tile_pool(name="ps_p", bufs=2, space="PSUM") as ps_p,
            tc.tile_pool(name="ps_t", bufs=1, space="PSUM") as ps_t,
        ):
            # ---- persistent SBUF state. All input DMAs are issued up front
            # (each dma_start costs ~650ns of issuing-engine time), spread
            # over four engine queues so issue serialization doesn't gate
            # the first tiles, in first-use order per queue. ----
            xres = const.tile([P, T // QS, CB, QS], BF16, tag="xres")
            xqr = const.tile([P, NSLOT, CB // 2, 2, QS], FP8, tag="xqr")
            mask8_sb = const.tile([P, 8, QS], BF16, tag="mask8")
            w2_sb = const.tile([P, CB, P], BF16, tag="w2")
            wq2_sb = const.tile([P, CB // 2, 2, P], FP8, tag="wq2")
            bq2_sb = const.tile([P, 1], F32, tag="bq2")

            # Ring FIFO order follows issue order, so the front of the
            # stream is exactly what the first ~15us of compute needs:
            # weights, x tile 0 (split in half), q slot 0, mask, then the
            # remaining x tiles in first-use order.
            # scalar queue: all the small weights, ahead of everything
            nc.scalar.dma_start(w2_sb[:], w2_d[:])
            nc.scalar.dma_start(wq2_sb[:], wq2_d[:])
            nc.scalar.dma_start(bq2_sb[:], bq2_d[:])
            # sync queue: kv x tiles + slot-0 q
            nc.sync.dma_start(xres[:, 0, :4, :], x_kv_d[0, :, :4, :])
            nc.sync.dma_start(xres[:, 0, 4:, :], x_kv_d[0, :, 4:, :])
            nc.sync.dma_start(xqr[:, 0], x_q_d[0])
            nc.sync.dma_start(xres[:, 1, :, :], x_kv_d[1])
            for tb in range(4, T // QS):
                nc.sync.dma_start(xres[:, tb, :, :], x_kv_d[tb])
            # gpsimd queue: mask (slot 0 is all masked pairs), two
            # mid-stream kv tiles, rest of x_q
            nc.gpsimd.dma_start(mask8_sb[:], mask8_d[:])
            nc.gpsimd.dma_start(xres[:, 2, :, :], x_kv_d[2])
            nc.gpsimd.dma_start(xres[:, 3, :, :], x_kv_d[3])
            for s in range(1, NSLOT):
                nc.gpsimd.dma_start(xqr[:, s], x_q_d[s])

            id_bf16 = const.tile([P, P], BF16, tag="idb")
            make_identity(nc, id_bf16[:])

            # kv_sb[p, pair, parity, col]: even key chunks store [kT; vT],
            # odd chunks store [vT; kT], so a score pair's two stationaries
            # are kv_sb[:H, i, 0, :] and kv_sb[H:, i, 1, :] — disjoint PE
            # row groups with no duplication DMA. qT rows 64-127 are
            # duplicated for free by the [Wq|Wq] stationary.
            kv_sb = const.tile([P, NB // 2, 2, P], BF16, tag="kv")
            qT_sb = const.tile([P, NSLOT * QS], BF16, tag="qT")
            vext_b = const.tile([P, NB, HE], BF16, tag="vextb")
            nc.vector.memset(vext_b[:, :, H:HE], 1.0)
            vext_f = const.tile([P, NCLEAN // 2, 2, HEP], FP8, tag="vextf")
            nc.vector.memset(vext_f[:, :, :, H:HE], 1.0)
            nc.vector.memset(vext_f[:, :, :, HE:HEP], 0.0)

            def proj_thunks(tb):
                st = {}

                def mk_mm(c):
                    def f():
                        if c == 0:
                            st["pp"] = ps_p.tile([P, 2, 2, P], F32, tag="proj", name="pp")
                        nc.tensor.matmul(
                            st["pp"][:], w2_sb[:, c, :], xres[:, tb, c, :],
                            start=(c == 0), stop=(c == CB - 1),
                        )
                    return f

                def kv_copy():
                    pp = st["pp"]
                    # even chunks (parity 0): straight copy, full 128 rows
                    nc.vector.tensor_copy(
                        kv_sb[:, 2 * tb : 2 * tb + 2, 0, :], pp[:, :, 0, :]
                    )
                    # odd chunks (parity 1): swap halves so kT lands in
                    # rows 64-127 and vT in rows 0-63
                    nc.vector.tensor_copy(
                        kv_sb[H:, 2 * tb : 2 * tb + 2, 1, :], pp[:H, :, 1, :]
                    )
                    nc.vector.tensor_copy(
                        kv_sb[:H, 2 * tb : 2 * tb + 2, 1, :], pp[H:, :, 1, :]
                    )

                return [mk_mm(c) for c in range(CB)] + [kv_copy]

            def vtr_thunks(tb):
                def mk_vtr(sx):
                    def f():
                        tk = tb * (QS // P) + sx
                        par = tk % 2
                        vtp = ps_t.tile([P, H], BF16, tag="tr")
                        if par == 0:
                            vsrc = kv_sb[H:, tk // 2, 0, :]
                            idm = id_bf16[H:, H:]
                        else:
                            vsrc = kv_sb[:H, tk // 2, 1, :]
                            idm = id_bf16[:H, :H]
                        nc.tensor.transpose(vtp[:], vsrc, idm)
                        nc.vector.tensor_copy(vext_b[:, tk, :H], vtp[:])
                        if tk < NCLEAN:
                            nc.vector.tensor_copy(
                                vext_f[:, tk // 2, tk % 2, :H], vtp[:]
                            )
                    return f

                return [mk_vtr(sx) for sx in range(QS // P)]

            def kv_thunks(tb):
                return proj_thunks(tb) + vtr_thunks(tb)

            def q_proj(s):
                qp = ps_p.tile([P, QS], F32, tag="proj")
                for cp in range(CB // 2):
                    nc.tensor.matmul(
                        qp[:], wq2_sb[:, cp, :, :], xqr[:, s, cp, :, :],
                        start=(cp == 0), stop=(cp == CB // 2 - 1),
                        perf_mode=mybir.MatmulPerfMode.DoubleRow,
                    )
                qcols_ = slice(s * QS, (s + 1) * QS)
                nc.vector.tensor_scalar_add(qT_sb[:, qcols_], qp[:], bq2_sb[:])

            # tile 0 + q slot 0 gate the first score pair (pairs 0-1 only
            # touch key chunks 0-3 = tile 0); tile 1 rides in the first
            # fill region so the PE isn't queued behind its DMA
            for th in proj_thunks(0) + vtr_thunks(0):
                th()
            q_proj(0)

            # fill regions (thunks, first pair, deadline pair): each region
            # is dispatched evenly over its global-pair window
            regions = [
                (kv_thunks(1), 0, 2),
                (kv_thunks(2) + kv_thunks(3), 2, 8),
                (kv_thunks(4) + kv_thunks(5), 8, 19),
                (kv_thunks(6) + kv_thunks(7), 19, 31),
            ]
            rfill = [0] * len(regions)

            def run_fill(g):
                for r, (th, g0, g1) in enumerate(regions):
                    if g < g0:
                        continue
                    want = len(th) if g >= g1 else ((g - g0 + 1) * len(th)) // (g1 - g0)
                    while rfill[r] < want:
                        th[rfill[r]]()
                        rfill[r] += 1

            gpair = 0
            for s in range(NSLOT):
                U = SLOT_U[s]
                npairs = U // 2
                qcols = slice(s * QS, (s + 1) * QS)

                oacc = ps_o.tile([HEP, QS], F32, tag="outT")
                pipe = []  # (et, tkp, masked) awaiting their wv matmuls

                def emit_wv(et, tkp, masked, _U=U, _oacc=oacc):
                    first = tkp == 0
                    last = tkp == _U // 2 - 1
                    if masked:
                        for h in range(2):
                            tk = 2 * tkp + h
                            nc.tensor.matmul(
                                _oacc[:HE, :], vext_b[:, tk, :], et[:, h, :],
                                start=(first and h == 0), stop=(last and h == 1),
                            )
                    else:
                        nc.tensor.matmul(
                            _oacc[:], vext_f[:, tkp, :, :], et[:],
                            start=first, stop=last,
                            perf_mode=mybir.MatmulPerfMode.DoubleRow,
                        )

                for tkp in range(npairs):
                    masked = tkp >= npairs - 4
                    sps = ps_s.tile([P, 2, QS], F32, tag="sT")
                    nc.tensor.matmul(
                        sps[:, 0, :], kv_sb[:H, tkp, 0, :], qT_sb[:H, qcols],
                        start=True, stop=True,
                    )
                    nc.tensor.matmul(
                        sps[:, 1, :], kv_sb[H:, tkp, 1, :], qT_sb[H:, qcols],
                        start=True, stop=True,
                    )
                    if masked:
                        et = eb.tile([P, 2, QS], BF16, tag="expb")
                    else:
                        et = ef.tile([P, 2, QS], FP8, tag="expf")
                    nc.scalar.activation(
                        et[:], sps[:], mybir.ActivationFunctionType.Exp,
                        scale=float(H) ** -0.5 / QSCALE,
                    )
                    if masked:
                        mi = 2 * (tkp - (npairs - 4))
                        nc.vector.tensor_tensor(
                            et[:], et[:], mask8_sb[:, mi : mi + 2, :],
                            mybir.AluOpType.mult,
                        )
                    run_fill(gpair)
                    gpair += 1
                    # next slot's q projection early, so the slot boundary
                    # never waits on qT
                    if tkp == 1 and s < NSLOT - 1:
                        q_proj(s + 1)
                    # wv runs one pair behind scores so PE never stalls on ACT
                    pipe.append((et, tkp, masked))
                    if len(pipe) > 1:
                        emit_wv(*pipe.pop(0))
                while pipe:
                    emit_wv(*pipe.pop(0))

                ot = wout.tile([HE, QS], F32, tag="oT")
                nc.vector.tensor_copy(ot[:], oacc[:HE, :])
                nc.gpsimd.dma_start(out_d[s], ot[:])

    nc.compile()
    return nc


_NC_CACHE = None


def _get_nc():
    global _NC_CACHE
    if _NC_CACHE is None:
        _NC_CACHE = build_bass()
    return _NC_CACHE


def _core_inputs(x, Wq, bq, Wk, bk, Wv, bv, b, fold):
    xT = np.asarray(x[b], dtype=np.float32).T          # [C, T]
    x_kv = np.ascontiguousarray(
        xT.reshape(CB, P, T // QS, QS).transpose(2, 1, 0, 3).astype(BF16NP)
    )
    qcols = np.concatenate([_qcols(fold, s) for s in range(NSLOT)])
    xq = xT[:, qcols]                                   # [C, 2048]
    x_q = np.ascontiguousarray(
        xq.reshape(CB // 2, 2, P, NSLOT, QS).transpose(3, 2, 0, 1, 4).astype(FP8NP)
    )
    wk = np.asarray(Wk, np.float32).reshape(CB, P, H)
    wv = np.asarray(Wv, np.float32).reshape(CB, P, H)
    w2 = np.ascontiguousarray(
        np.concatenate([wk, wv], axis=2).transpose(1, 0, 2).astype(BF16NP)
    )
    wqs = (QSCALE * np.asarray(Wq, np.float32)).reshape(CB // 2, 2, P, H)
    wq2 = np.ascontiguousarray(
        np.concatenate([wqs, wqs], axis=3).transpose(2, 0, 1, 3).astype(FP8NP)
    )
    bq2 = np.ascontiguousarray(
        np.tile(QSCALE * np.asarray(bq, np.float32), 2)[:, None]
    )
    p = np.arange(P)[:, None, None]
    i = np.arange(8)[None, :, None]
    j = np.arange(QS)[None, None, :]
    if fold == 0:
        m = (448 + j) >= (128 * i + p)
    else:
        m = np.where(j < 448, j >= (128 * i + p), (512 + j) >= (128 * i + p))
    mask8 = np.ascontiguousarray(m.astype(BF16NP))
    return {
        "x_kv": x_kv,
        "x_q": x_q,
        "w2": w2,
        "wq2": wq2,
        "bq2": bq2,
        "mask8": mask8,
    }


def _assemble(results, bv):
    bvf = np.asarray(bv, np.float32)
    out = np.empty((B, T, H), dtype=np.float32)
    for core in range(8):
        b, fold = core // 2, core % 2
        o = results[core]["out"]                        # [NSLOT, 65, 512]
        for s in range(NSLOT):
            val = (o[s, :H, :] / o[s, H:H + 1, :]).T + bvf
            out[b, _qcols(fold, s), :] = val
    return out


def kernel(x, Wq, bq, Wk, bk, Wv, bv):
    x = np.asarray(x, dtype=np.float32)
    nc = _get_nc()
    core_ids = list(range(8))
    in_maps = [
        _core_inputs(x, Wq, bq, Wk, bk, Wv, bv, core // 2, core % 2)
        for core in core_ids
    ]
    res = run_bass_kernel_spmd(nc, in_maps, core_ids)
    return _assemble(res.results, bv)
